# revision 1
# baseline (speedup 1.0000x reference)
"""Trainium2 kernel for nn_Attention_local_4088808866313 (sparse windowed attention).

Sharding: data-parallel over batch b (8 cores, one batch element each).
Device per core: x-transpose (PE), depthwise 5x5 conv + folded BN for q/k/v
(PE diagonal matmuls accumulating 25 taps in PSUM), top-8 routing over
gen_adj rows (DVE Max8/MaxIndex).  Host: windowed gather + tiny-matmul
attention (numpy), which is irregular/data-dependent.
"""

import os
import numpy as np

B, L, D = 8, 1024, 768
HEADS, DH = 16, 48
H = W = 32
H2 = W2 = 16
P2 = 256
K = 8
PW = 36  # padded image side (32 + 2*2)
EPS = 1e-5

LAST_EXEC_NS = None


def _build_program():
    from concourse import bacc, mybir
    import concourse.tile as tile
    from concourse.masks import make_identity

    nc = bacc.Bacc("TRN2", target_bir_lowering=False)
    f32 = mybir.dt.float32

    x_in = nc.dram_tensor("x_in", [L, D], f32, kind="ExternalInput")
    adj_in = nc.dram_tensor("adj_in", [HEADS * P2, P2], f32, kind="ExternalInput")
    dw_in = nc.dram_tensor("dw_in", [18, 25, 128, 32], f32, kind="ExternalInput")
    bias_in = nc.dram_tensor("bias_in", [128, 18], f32, kind="ExternalInput")
    qkv_out = nc.dram_tensor("qkv_out", [3, D, L], f32, kind="ExternalOutput")
    idx_out = nc.dram_tensor(
        "idx_out", [HEADS * P2, K], mybir.dt.uint32, kind="ExternalOutput"
    )

    with tile.TileContext(nc) as tc:
        with (
            tc.tile_pool(name="const", bufs=1) as constp,
            tc.tile_pool(name="xload", bufs=3) as xp,
            tc.tile_pool(name="imgs", bufs=1) as imgp,
            tc.tile_pool(name="wpool", bufs=6) as wp,
            tc.tile_pool(name="outp", bufs=3) as op,
            tc.tile_pool(name="adjp", bufs=3) as adjp,
            tc.tile_pool(name="tkp", bufs=3) as tkp,
            tc.tile_pool(name="pst", bufs=2, space="PSUM") as pst,
            tc.tile_pool(name="psc", bufs=2, space="PSUM") as psc,
        ):
            ident = constp.tile([128, 128], f32, tag="ident")
            make_identity(nc, ident[:])
            bias_sb = constp.tile([128, 18], f32, tag="bias")
            nc.sync.dma_start(bias_sb[:], bias_in[:])

            # --- top-8 routing (independent of conv; overlaps) ---
            for i in range(HEADS * P2 // 128):
                at = adjp.tile([128, P2], f32, tag="adj")
                nc.sync.dma_start(at[:], adj_in[i * 128 : (i + 1) * 128, :])
                mx = tkp.tile([128, 8], f32, tag="mx")
                ix = tkp.tile([128, 8], mybir.dt.uint32, tag="ix")
                nc.vector.max_with_indices(mx[:], ix[:], at[:])
                nc.sync.dma_start(idx_out[i * 128 : (i + 1) * 128, :], ix[:])

            # --- padded channel-major images (zero halo) ---
            imgs = []
            for ct in range(6):
                t = imgp.tile([128, PW * PW], f32, tag=f"img{ct}")
                nc.gpsimd.memset(t[:], 0.0)
                imgs.append(t)

            # --- transpose x (L,D) -> channel-major padded images ---
            for pt in range(8):
                xt = xp.tile([128, D], f32, tag="xt")
                nc.sync.dma_start(xt[:], x_in[pt * 128 : (pt + 1) * 128, :])
                for ct in range(6):
                    ps = pst.tile([128, 128], f32, tag="pst")
                    nc.tensor.transpose(
                        ps[:], xt[:, ct * 128 : (ct + 1) * 128], ident[:]
                    )
                    # pixel rows 4*pt .. 4*pt+3 into padded layout
                    dv = imgs[ct][:].rearrange("p (a b) -> p a b", a=PW)[
                        :, 2 + 4 * pt : 6 + 4 * pt, 2:34
                    ]
                    sv = ps[:].rearrange("p (a b) -> p a b", a=4)
                    nc.vector.tensor_copy(dv, sv)

            # --- depthwise conv as 25 diagonal matmuls accumulated in PSUM ---
            for ct in range(6):
                img3 = None
                for j in range(3):
                    pc = psc.tile([128, 1024], f32, tag="pc")
                    for t in range(25):
                        dy, dx = t // 5, t % 5
                        wt = wp.tile([128, 32], f32, tag="wt")
                        nc.sync.dma_start(wt[:], dw_in[j * 6 + ct, t, :, :])
                        img3 = imgs[ct][:].rearrange("p (a b) -> p a b", a=PW)
                        for g in range(4):
                            sl = slice(32 * g, 32 * g + 32)
                            rhs1 = img3[sl, dy : dy + 16, dx : dx + 32]
                            rhs2 = img3[sl, dy + 16 : dy + 32, dx : dx + 32]
                            nc.tensor.matmul(
                                pc[sl, :512],
                                wt[sl, :],
                                rhs1,
                                start=(t == 0),
                                stop=(t == 24),
                                tile_position=(32 * g, 32 * g),
                            )
                            nc.tensor.matmul(
                                pc[sl, 512:],
                                wt[sl, :],
                                rhs2,
                                start=(t == 0),
                                stop=(t == 24),
                                tile_position=(32 * g, 32 * g),
                            )
                    ob = op.tile([128, 1024], f32, tag="ob")
                    nc.scalar.activation(
                        ob[:],
                        pc[:],
                        mybir.ActivationFunctionType.Identity,
                        bias=bias_sb[:, j * 6 + ct : j * 6 + ct + 1],
                        scale=1.0,
                    )
                    nc.sync.dma_start(
                        qkv_out[j, ct * 128 : (ct + 1) * 128, :], ob[:]
                    )
    nc.finalize()
    return nc


def _fold_weights(conv_w, bn_gamma, bn_beta, bn_mean, bn_var):
    # conv_w: (3, 768, 1, 5, 5)
    inv = bn_gamma / np.sqrt(bn_var + EPS)  # (3, 768)
    w_eff = conv_w[:, :, 0, :, :] * inv[:, :, None, None]  # (3, 768, 5, 5)
    b_eff = bn_beta - bn_mean * inv  # (3, 768)
    scale = float(D) ** -0.5
    w_eff = w_eff.copy()
    b_eff = b_eff.copy()
    w_eff[0] *= scale  # fold q scaling
    b_eff[0] *= scale
    dw = np.zeros((18, 25, 128, 32), np.float32)
    ar = np.arange(128)
    for j in range(3):
        for ct in range(6):
            blk = w_eff[j, ct * 128 : (ct + 1) * 128].reshape(128, 25)
            for t in range(25):
                dw[j * 6 + ct, t, ar, ar % 32] = blk[:, t]
    bias = np.zeros((128, 18), np.float32)
    for j in range(3):
        for ct in range(6):
            bias[:, j * 6 + ct] = b_eff[j, ct * 128 : (ct + 1) * 128]
    return dw, bias


def _windowify(t):
    # t: (n, H, W, c) -> (n, H2*W2, 4, c)
    n, HH, WW, c = t.shape
    h2, w2 = HH // 2, WW // 2
    t = t.reshape(n, 2, h2, 2, w2, c).transpose(0, 2, 4, 1, 3, 5)
    return t.reshape(n, h2 * w2, 4, c)


def _host_finish(qkv, idxs):
    # qkv: (B, 3, 768, 1024) f32; idxs: (B, 4096, 8)
    b, heads, dh = B, HEADS, DH
    bh = b * heads

    def to_pix(t):  # (B, D, L) -> (bh, p2, 4, dh), windowified
        t = t.reshape(b, heads, dh, H * W).transpose(0, 1, 3, 2)
        return _windowify(t.reshape(bh, H, W, dh))

    q_pix = to_pix(qkv[:, 0])
    k_pix = to_pix(qkv[:, 1])
    v_pix = to_pix(qkv[:, 2])

    r_idx = idxs.reshape(bh, P2, K).astype(np.intp)
    ar = np.arange(bh)[:, None, None]
    k_sel = k_pix[ar, r_idx]  # (bh, p2, K, 4, dh)
    v_sel = v_pix[ar, r_idx]  # (bh, p2, K, 4, dh)

    # q already scaled by D**-0.5 on device (folded into conv weights)
    logits = np.einsum(
        "npqc,npemc->npqem", q_pix, k_sel, optimize=True
    ).reshape(bh, P2, 4, K * 4)
    logits -= logits.max(axis=-1, keepdims=True)
    e = np.exp(logits)
    attn = e / e.sum(axis=-1, keepdims=True)
    o = np.einsum(
        "npqk,npkc->npqc",
        attn,
        v_sel.reshape(bh, P2, K * 4, dh),
        optimize=True,
    )
    # o dims after reshape: (b, head, r, s, i, j, c); ref order: (b, j, r, i, s, head, c)
    o = o.reshape(b, heads, H2, W2, 2, 2, dh).transpose(0, 5, 2, 4, 3, 1, 6)
    o = o.reshape(b, H, W, heads * dh)
    return np.ascontiguousarray(o.reshape(b, H * W, D).astype(np.float32))


def kernel(x, noise, gen_adj, conv_w, bn_gamma, bn_beta, bn_mean, bn_var, sparsity):
    global LAST_EXEC_NS
    from concourse.bass_utils import run_bass_kernel_spmd

    assert int(sparsity) == K
    x = np.asarray(x, np.float32)
    gen_adj = np.asarray(gen_adj, np.float32)
    dw, bias = _fold_weights(
        np.asarray(conv_w, np.float32),
        np.asarray(bn_gamma, np.float32),
        np.asarray(bn_beta, np.float32),
        np.asarray(bn_mean, np.float32),
        np.asarray(bn_var, np.float32),
    )

    nc = _build_program()
    in_maps = []
    for bb in range(B):
        in_maps.append(
            {
                "x_in": np.ascontiguousarray(x[bb]),
                "adj_in": np.ascontiguousarray(
                    gen_adj[bb].reshape(HEADS * P2, P2)
                ),
                "dw_in": dw,
                "bias_in": bias,
            }
        )

    trace = os.environ.get("KERNEL_TRACE", "0") == "1"
    res = run_bass_kernel_spmd(
        nc, in_maps, core_ids=list(range(B)), trace=trace
    )
    if trace:
        LAST_EXEC_NS = res.exec_time_ns
    if os.environ.get("KERNEL_TIME", "0") == "1":
        # second run hits the in-process PJRT executable cache; wall-time it
        import time as _time

        t0 = _time.time()
        res = run_bass_kernel_spmd(
            nc, in_maps, core_ids=list(range(B)), trace=False
        )
        LAST_EXEC_NS = int((_time.time() - t0) * 1e9)

    qkv = np.stack([r["qkv_out"] for r in res.results])  # (B, 3, 768, 1024)
    idxs = np.stack([r["idx_out"] for r in res.results])  # (B, 4096, 8)
    return _host_finish(qkv, idxs)


if __name__ == "__main__":
    rng = np.random.default_rng(0)
    inputs = {
        "x": rng.standard_normal((B, L, D), dtype=np.float32),
        "noise": np.zeros((1,), np.float32),
        "gen_adj": rng.standard_normal((B, HEADS, P2, P2), dtype=np.float32),
        "conv_w": (rng.standard_normal((3, D, 1, 5, 5)) * 0.1).astype(np.float32),
        "bn_gamma": (1.0 + 0.1 * rng.standard_normal((3, D))).astype(np.float32),
        "bn_beta": (0.1 * rng.standard_normal((3, D))).astype(np.float32),
        "bn_mean": (0.1 * rng.standard_normal((3, D))).astype(np.float32),
        "bn_var": rng.uniform(0.5, 1.5, (3, D)).astype(np.float32),
        "sparsity": 8,
    }
    out = kernel(**inputs)
    print(out.shape, out.dtype, float(np.abs(out).max()))



# revision 14
# speedup vs baseline: 3.9475x; 3.9475x over previous
"""Trainium2 kernel for nn_Attention_local_4088808866313 (sparse windowed attention).

Sharding: data-parallel over batch b (8 cores, one batch element each).
Device per core: full pipeline — x transpose (xbar DMA), depthwise 5x5
conv + folded BN (PE diagonal matmuls), masked dense attention per head
(QK^T with an additive -30 window-mask bias folded into the PSUM
accumulation as a second matmul, exp on ACT, AV with an appended
ones-column in V to produce softmax denominators), normalization and
the output quadrant permutation. Host: BN fold + top-8 window routing
(numpy argpartition) shipped as indices; mask matrices are built
on-device from the indices with match_replace.
"""

import os
import numpy as np
import ml_dtypes

BF = ml_dtypes.bfloat16

B, L, D = 8, 1024, 768
HEADS, DH = 16, 48
H = W = 32
H2 = W2 = 16
P2 = 256
K = 8
PW = 36  # padded image side (32 + 2*2)
EPS = 1e-5
NEG = -30.0  # additive mask bias; exp(-30) ~ 1e-13, negligible vs selected keys

LAST_EXEC_NS = None

# consts blob column layout (bf16 [128, 640])
C_IOTA = 0    # [128, 256] each row = 0..255
C_ID = 256    # [128, 128] identity
C_REVEN = 384  # [128, 128] R_even[k,p] = (k == 16*(p//32) + p%16)
C_RODD = 512   # [128, 128] R_odd[k,p]  = (k == 64 + 16*(p//32) + p%16)


def _emit_device(nc, x_in, w_in, bias_in, idx_in, consts_in, o_out, dbg=None):
    from concourse import mybir
    import concourse.bass as bass
    import concourse.tile as tile

    F32 = mybir.dt.float32
    BF16 = mybir.dt.bfloat16
    tc_ctx = tile.TileContext(nc)
    with tc_ctx as tc:
        with (
            tc.tile_pool(name="const", bufs=1) as constp,
            tc.tile_pool(name="chm", bufs=1) as chmp,      # Q/K/V persistent
            tc.tile_pool(name="imgs", bufs=1) as imgp,
            tc.tile_pool(name="xload", bufs=2) as xp,
            tc.tile_pool(name="dwp", bufs=3) as dwp,
            tc.tile_pool(name="maskp", bufs=3) as maskp,
            tc.tile_pool(name="bwtp", bufs=1) as bwtp,
            tc.tile_pool(name="ptp", bufs=8) as ptp,
            tc.tile_pool(name="otp", bufs=2) as otp,
            tc.tile_pool(name="finp", bufs=2) as finp,
            tc.tile_pool(name="pcp", bufs=1, space="PSUM") as pcp,
            tc.tile_pool(name="pss", bufs=1, space="PSUM") as pssp,
            tc.tile_pool(name="pso", bufs=1, space="PSUM") as psop,
            tc.tile_pool(name="pst", bufs=2, space="PSUM") as pstp,
        ):
            # ---------------- constants / small inputs ----------------
            consts = constp.tile([128, 640], BF16, tag="consts")
            nc.sync.dma_start(consts[:], consts_in[:])
            w_sb = constp.tile([128, 600], F32, tag="w_sb")
            nc.sync.dma_start(w_sb[:], w_in[:])
            bias_sb = constp.tile([128, 24], F32, tag="bias_sb")
            nc.sync.dma_start(bias_sb[:], bias_in[:])
            idx_sb = constp.tile([128, 256], BF16, tag="idx_sb")
            nc.sync.dma_start(idx_sb[:], idx_in[:])

            iotaM = consts[:, C_IOTA:C_IOTA + 256]
            ident = consts[:, C_ID:C_ID + 128]
            r_even = consts[:, C_REVEN:C_REVEN + 128]
            r_odd = consts[:, C_RODD:C_RODD + 128]

            # ---------------- mask build: bwT[k2] [128 kw, 16h*256 qw] ----------
            bwT = [bwtp.tile([128, 4096], BF16, tag=f"bwt{k2}", name=f"bwt{k2}") for k2 in range(2)]
            for h in range(HEADS):
                for i in range(2):
                    a = 2 * h + i
                    m1 = maskp.tile([128, 256], BF16, tag="m1")
                    nc.vector.match_replace(
                        m1[:], idx_sb[:, 8 * a:8 * a + 8], iotaM, -1.0
                    )
                    bw = maskp.tile([128, 256], BF16, tag="bw")
                    # selected -> 0, not selected -> NEG
                    nc.vector.tensor_scalar(
                        bw[:], m1[:], 0.0, NEG,
                        op0=mybir.AluOpType.is_ge, op1=mybir.AluOpType.mult,
                    )
                    for k2 in range(2):
                        psB = pstp.tile([128, 128], BF16, tag="pst")
                        nc.tensor.transpose(
                            psB[:], bw[:, 128 * k2:128 * k2 + 128], ident
                        )
                        nc.vector.tensor_copy(
                            bwT[k2][:, h * 256 + 128 * i: h * 256 + 128 * i + 128],
                            psB[:],
                        )

            if dbg is not None:
                nc.sync.dma_start(dbg["bwt0"][:], bwT[0][:])
                nc.sync.dma_start(dbg["bwt1"][:], bwT[1][:])

            # ---------------- x load: transpose to padded channel-major ---------
            imgs = []
            for b in range(8):
                img = imgp.tile([128, PW * PW], BF16, tag=f"img{b}")
                nc.gpsimd.memset(img[:], 0.0)
                imgs.append(img)
            for b in range(8):
                xt = xp.tile([128, 1024], BF16, tag="xt")
                nc.gpsimd.memset(xt[:], 0.0)
                nc.sync.dma_start_transpose(
                    xt[0:48, :], x_in[:, 96 * b:96 * b + 48]
                )
                nc.sync.dma_start_transpose(
                    xt[64:112, :], x_in[:, 96 * b + 48:96 * b + 96]
                )
                dv = imgs[b][:].rearrange("p (a b) -> p a b", a=PW)[:, 2:34, 2:34]
                sv = xt[:].rearrange("p (a b) -> p a b", a=H)
                nc.vector.tensor_copy(dv, sv)

            # ---------------- conv: diag matmuls, 25 taps in PSUM ----------
            q_chm, k_chm = [], []
            v_pix = [chmp.tile([128, 1024], BF16, tag=f"vpix{c8}", name=f"vpix{c8}") for c8 in range(8)]
            for b in range(8):
                img3 = imgs[b][:].rearrange("p (a b) -> p a b", a=PW)
                for j in range(3):
                    pc = pcp.tile([128, 1024], F32, tag="pc")
                    for t in range(25):
                        dy, dx = t // 5, t % 5
                        dw = dwp.tile([128, 128], BF16, tag="dw")
                        col = b * 75 + j * 25 + t
                        nc.vector.tensor_scalar(
                            dw[:], ident, w_sb[:, col:col + 1], None,
                            op0=mybir.AluOpType.mult,
                        )
                        nc.tensor.matmul(
                            pc[:, :512], dw[:],
                            img3[:, dy:dy + 16, dx:dx + 32],
                            start=(t == 0), stop=(t == 24),
                        )
                        nc.tensor.matmul(
                            pc[:, 512:], dw[:],
                            img3[:, dy + 16:dy + 32, dx:dx + 32],
                            start=(t == 0), stop=(t == 24),
                        )
                    bcol = b * 3 + j
                    dst = chmp.tile([128, 1024], BF16, tag=f"chm{j}_{b}")
                    nc.scalar.activation(
                        dst[:], pc[:],
                        mybir.ActivationFunctionType.Identity,
                        bias=bias_sb[:, bcol:bcol + 1], scale=1.0,
                    )
                    if j == 0:
                        q_chm.append(dst)
                    elif j == 1:
                        k_chm.append(dst)
                    else:
                        # V: rows 48/112 are all-ones via the conv bias
                        # (zero weights + bias=1) -> softmax denominator
                        for c8 in range(8):
                            psT = pstp.tile([128, 128], BF16, tag="pst")
                            nc.tensor.transpose(
                                psT[:], dst[:, 128 * c8:128 * c8 + 128], ident
                            )
                            nc.vector.tensor_copy(
                                v_pix[c8][:, 128 * b:128 * b + 128], psT[:]
                            )

            if dbg is not None:
                nc.sync.dma_start(dbg["qchm0"][:], q_chm[0][:])
                nc.sync.dma_start(dbg["kchm0"][:], k_chm[0][:])
                nc.sync.dma_start(dbg["vpix0"][:], v_pix[0][:])

            # ---------------- attention per head ----------------
            out_pix = [chmp.tile([128, 784], BF16, tag=f"opix{c8}", name=f"opix{c8}") for c8 in range(8)]
            for h in range(HEADS):
                bh = h // 2
                p0 = 64 * (h % 2)
                pts = []
                for c in range(8):
                    cp = c % 4
                    k2 = cp // 2
                    rmat = r_even if (cp % 2 == 0) else r_odd
                    psS = pssp.tile([128, 1024], F32, tag="pss")
                    for n0 in range(2):
                        nc.tensor.matmul(
                            psS[:, 512 * n0:512 * n0 + 512],
                            k_chm[bh][p0:p0 + 64, 128 * c:128 * c + 128],
                            q_chm[bh][p0:p0 + 64, 512 * n0:512 * n0 + 512],
                            start=True, stop=False,
                        )
                        # additive window-mask bias: rhs = bwT with the
                        # (qw -> qpx) expansion AP [(rq,16),(jq,0x2),(sq,1)]
                        sl = bwT[k2][:, h * 256:h * 256 + 256]
                        rhs = bass.AP(
                            tensor=sl.tensor,
                            offset=sl.offset,
                            ap=[list(p) for p in sl.ap[:1]]
                            + [[16, 16], [0, 2], [1, 16]],
                        )
                        nc.tensor.matmul(
                            psS[:, 512 * n0:512 * n0 + 512],
                            rmat, rhs, start=False, stop=True,
                        )
                    pt = ptp.tile([128, 1024], BF16, tag="pt")
                    nc.scalar.activation(
                        pt[:], psS[:], mybir.ActivationFunctionType.Exp,
                        bias=0.0, scale=1.0,
                    )
                    pts.append(pt)
                    if dbg is not None and h == 0 and c == 0:
                        nc.sync.dma_start(dbg["pt00"][:], pt[:])
                psO = psop.tile([128, 1024], F32, tag="pso")
                for n0 in range(2):
                    for c in range(8):
                        nc.tensor.matmul(
                            psO[0:49, 512 * n0:512 * n0 + 512],
                            v_pix[c][:, 128 * bh + p0:128 * bh + p0 + 49],
                            pts[c][:, 512 * n0:512 * n0 + 512],
                            start=(c == 0), stop=(c == 7),
                        )
                oT = otp.tile([128, 1024], BF16, tag="ot")
                nc.vector.tensor_copy(oT[0:49, :], psO[0:49, :])
                if dbg is not None and h == 0:
                    nc.sync.dma_start(dbg["ot0"][:], oT[0:49, :])
                for c8 in range(8):
                    psT2 = pstp.tile([128, 128], BF16, tag="pst")
                    nc.tensor.transpose(
                        psT2[:, 0:49],
                        oT[0:49, 128 * c8:128 * c8 + 128],
                        ident[0:49, 0:49],
                    )
                    nc.vector.tensor_copy(
                        out_pix[c8][:, 49 * h:49 * h + 49], psT2[:, 0:49]
                    )

            # ---------------- normalize + quadrant-permuted store ----------
            for c8 in range(8):
                cs = finp.tile([128, 16], F32, tag="cs")
                nc.vector.tensor_copy(cs[:], out_pix[c8][:, 48:784:49])
                rc = finp.tile([128, 16], F32, tag="rc")
                nc.vector.reciprocal(rc[:], cs[:])
                fin = finp.tile([128, 768], BF16, tag="fin")
                for h in range(HEADS):
                    nc.vector.tensor_scalar(
                        fin[:, 48 * h:48 * h + 48],
                        out_pix[c8][:, 49 * h:49 * h + 48],
                        rc[:, h:h + 1], None,
                        op0=mybir.AluOpType.mult,
                    )
                nc.sync.dma_start(
                    o_out[128 * c8:128 * c8 + 128, :], fin[:]
                )
    return nc


def _build_program():
    from concourse import bacc, mybir

    nc = bacc.Bacc("TRN2", target_bir_lowering=False)
    F32 = mybir.dt.float32
    BF16 = mybir.dt.bfloat16

    x_in = nc.dram_tensor("x_in", [L, D], BF16, kind="ExternalInput")
    w_in = nc.dram_tensor("w_in", [128, 600], F32, kind="ExternalInput")
    bias_in = nc.dram_tensor("bias_in", [128, 24], F32, kind="ExternalInput")
    idx_in = nc.dram_tensor("idx_in", [128, 256], BF16, kind="ExternalInput")
    consts_in = nc.dram_tensor("consts_in", [128, 640], BF16, kind="ExternalInput")
    o_out = nc.dram_tensor("o_out", [L, D], BF16, kind="ExternalOutput")

    _emit_device(nc, x_in, w_in, bias_in, idx_in, consts_in, o_out)
    nc.finalize()
    return nc


def _chan_of(b, p):
    if p < 48:
        return 96 * b + p
    if 64 <= p < 112:
        return 96 * b + 48 + (p - 64)
    return -1


def _host_prepare(conv_w, bn_gamma, bn_beta, bn_mean, bn_var):
    inv = bn_gamma / np.sqrt(bn_var + EPS)  # (3, 768)
    w_eff = conv_w[:, :, 0, :, :] * inv[:, :, None, None]  # (3, 768, 5, 5)
    b_eff = bn_beta - bn_mean * inv  # (3, 768)
    scale = float(D) ** -0.5
    w_eff = w_eff.copy()
    b_eff = b_eff.copy()
    w_eff[0] *= scale
    b_eff[0] *= scale

    w600 = np.zeros((128, 600), np.float32)
    bias24 = np.zeros((128, 24), np.float32)
    for b in range(8):
        for p in range(128):
            ch = _chan_of(b, p)
            if ch < 0:
                continue
            for j in range(3):
                w600[p, b * 75 + j * 25: b * 75 + j * 25 + 25] = w_eff[j, ch].reshape(25)
                bias24[p, b * 3 + j] = b_eff[j, ch]
        # V ones-rows (pad rows 48/112 have zero weights): bias 1.0 makes the
        # conv emit constant 1.0 there -> softmax denominator column in AV
        bias24[48, b * 3 + 2] = 1.0
        bias24[112, b * 3 + 2] = 1.0

    consts = np.zeros((128, 640), np.float32)
    consts[:, C_IOTA:C_IOTA + 256] = np.arange(256)[None, :]
    consts[:, C_ID:C_ID + 128] = np.eye(128)
    p = np.arange(128)
    kloc = 16 * (p // 32) + p % 16
    consts[:, C_REVEN:C_REVEN + 128] = (np.arange(128)[:, None] == kloc[None, :])
    consts[:, C_RODD:C_RODD + 128] = (np.arange(128)[:, None] == (64 + kloc)[None, :])
    return w600, bias24, consts.astype(BF)


def _topk_idx(gen_adj):
    # (nb, 16, 256, 256) -> idx_sb (nb, 128, 256) bf16 (set semantics; order free)
    nb = gen_adj.shape[0]
    flat = gen_adj.reshape(nb * HEADS * P2, P2)
    part = np.argpartition(flat, P2 - K, axis=-1)[:, P2 - K:]  # (nb*H*P2, 8)
    idx4 = part.reshape(nb, HEADS, 2, 128, K).transpose(0, 3, 1, 2, 4)
    return np.ascontiguousarray(idx4.reshape(nb, 128, 256)).astype(BF)


def kernel(x, noise, gen_adj, conv_w, bn_gamma, bn_beta, bn_mean, bn_var, sparsity):
    global LAST_EXEC_NS
    from concourse.bass_utils import run_bass_kernel_spmd

    assert int(sparsity) == K
    x = np.asarray(x, np.float32)
    gen_adj = np.asarray(gen_adj, np.float32)
    w600, bias24, consts = _host_prepare(
        np.asarray(conv_w, np.float32),
        np.asarray(bn_gamma, np.float32),
        np.asarray(bn_beta, np.float32),
        np.asarray(bn_mean, np.float32),
        np.asarray(bn_var, np.float32),
    )
    idx_sb = _topk_idx(gen_adj)
    x_bf = x.astype(BF)

    nc = _build_program()
    in_maps = []
    for bb in range(B):
        in_maps.append(
            {
                "x_in": np.ascontiguousarray(x_bf[bb]),
                "w_in": w600,
                "bias_in": bias24,
                "idx_in": np.ascontiguousarray(idx_sb[bb]),
                "consts_in": consts,
            }
        )

    trace = os.environ.get("KERNEL_TRACE", "0") == "1"
    res = run_bass_kernel_spmd(
        nc, in_maps, core_ids=list(range(B)), trace=trace
    )
    if trace:
        LAST_EXEC_NS = res.exec_time_ns
    if os.environ.get("KERNEL_TIME", "0") == "1":
        # second run hits the in-process PJRT executable cache; wall-time it
        import time as _time

        t0 = _time.time()
        res = run_bass_kernel_spmd(
            nc, in_maps, core_ids=list(range(B)), trace=False
        )
        LAST_EXEC_NS = int((_time.time() - t0) * 1e9)

    o = np.stack([np.asarray(r["o_out"], np.float32) for r in res.results])
    # quadrant permutation: out pixel (jq*16+r, iq*16+s) <- device row (iq*16+r, jq*16+s)
    o = o.reshape(B, 2, 16, 2, 16, D).transpose(0, 3, 2, 1, 4, 5)
    return np.ascontiguousarray(o.reshape(B, L, D))


if __name__ == "__main__":
    rng = np.random.default_rng(0)
    inputs = {
        "x": rng.standard_normal((B, L, D), dtype=np.float32),
        "noise": np.zeros((1,), np.float32),
        "gen_adj": rng.standard_normal((B, HEADS, P2, P2), dtype=np.float32),
        "conv_w": (rng.standard_normal((3, D, 1, 5, 5)) * 0.1).astype(np.float32),
        "bn_gamma": (1.0 + 0.1 * rng.standard_normal((3, D))).astype(np.float32),
        "bn_beta": (0.1 * rng.standard_normal((3, D))).astype(np.float32),
        "bn_mean": (0.1 * rng.standard_normal((3, D))).astype(np.float32),
        "bn_var": rng.uniform(0.5, 1.5, (3, D)).astype(np.float32),
        "sparsity": 8,
    }
    out = kernel(**inputs)
    print(out.shape, out.dtype, float(np.abs(out).max()))


# revision 20
# speedup vs baseline: 4.6176x; 1.1698x over previous
"""Trainium2 kernel for nn_Attention_local_4088808866313 (sparse windowed attention).

Sharding: data-parallel over batch b (8 cores, one batch element each).
Device per core: full pipeline — x transpose (xbar DMA), depthwise 5x5
conv + folded BN (PE diagonal matmuls), masked dense attention per head
(QK^T with an additive -30 window-mask bias folded into the PSUM
accumulation as a second matmul, exp on ACT, AV with an appended
ones-column in V to produce softmax denominators), normalization and
the output quadrant permutation. Host: BN fold + top-8 window routing
(numpy argpartition) shipped as indices; mask matrices are built
on-device from the indices with match_replace.
"""

import os
import numpy as np
import ml_dtypes

BF = ml_dtypes.bfloat16

B, L, D = 8, 1024, 768
HEADS, DH = 16, 48
H = W = 32
H2 = W2 = 16
P2 = 256
K = 8
PW = 36  # padded image side (32 + 2*2)
EPS = 1e-5
NEG = -30.0  # additive mask bias; exp(-30) ~ 1e-13, negligible vs selected keys

LAST_EXEC_NS = None

# consts blob column layout (bf16 [128, 640])
C_IOTA = 0    # [128, 256] each row = 0..255
C_ID = 256    # [128, 128] identity
C_REVEN = 384  # [128, 128] R_even[k,p] = (k == 16*(p//32) + p%16)
C_RODD = 512   # [128, 128] R_odd[k,p]  = (k == 64 + 16*(p//32) + p%16)


def _emit_device(nc, x_in, w_in, bias_in, idx_in, consts_in, o_out, dbg=None, phase=3):
    from concourse import mybir
    import concourse.bass as bass
    import concourse.tile as tile

    F32 = mybir.dt.float32
    BF16 = mybir.dt.bfloat16
    tc_ctx = tile.TileContext(nc)
    with tc_ctx as tc:
        with (
            tc.tile_pool(name="const", bufs=1) as constp,
            tc.tile_pool(name="chm", bufs=1) as chmp,      # Q/K/V persistent
            tc.tile_pool(name="imgs", bufs=1) as imgp,
            tc.tile_pool(name="xload", bufs=2) as xp,
            tc.tile_pool(name="dwp", bufs=2) as dwp,
            tc.tile_pool(name="maskp", bufs=3) as maskp,
            tc.tile_pool(name="bwtp", bufs=1) as bwtp,
            tc.tile_pool(name="ptp", bufs=8) as ptp,
            tc.tile_pool(name="otp", bufs=2) as otp,
            tc.tile_pool(name="finp", bufs=2) as finp,
            tc.tile_pool(name="pcp", bufs=1, space="PSUM") as pcp,
            tc.tile_pool(name="pss", bufs=1, space="PSUM") as pssp,
            tc.tile_pool(name="pso", bufs=1, space="PSUM") as psop,
            tc.tile_pool(name="pst", bufs=2, space="PSUM") as pstp,
        ):
            # ---------------- constants / small inputs ----------------
            consts = constp.tile([128, 640], BF16, tag="consts")
            nc.sync.dma_start(consts[:], consts_in[:])
            w_sb = constp.tile([128, 600], BF16, tag="w_sb")
            nc.sync.dma_start(w_sb[:], w_in[:])
            bias_sb = constp.tile([128, 24], F32, tag="bias_sb")
            nc.sync.dma_start(bias_sb[:], bias_in[:])
            idx_sb = constp.tile([128, 256], BF16, tag="idx_sb")
            nc.sync.dma_start(idx_sb[:], idx_in[:])

            iotaM = consts[:, C_IOTA:C_IOTA + 256]
            ident = consts[:, C_ID:C_ID + 128]
            r_even = consts[:, C_REVEN:C_REVEN + 128]
            r_odd = consts[:, C_RODD:C_RODD + 128]

            # ---------------- mask build: bwT[k2] [128 kw, 16h*256 qw] ----------
            bwT = [bwtp.tile([128, 4096], BF16, tag=f"bwt{k2}", name=f"bwt{k2}") for k2 in range(2)]
            for h in range(HEADS if phase >= 2 else 0):
                for i in range(2):
                    a = 2 * h + i
                    m1 = maskp.tile([128, 256], BF16, tag="m1")
                    nc.vector.match_replace(
                        m1[:], idx_sb[:, 8 * a:8 * a + 8], iotaM, -1.0
                    )
                    bw = maskp.tile([128, 256], BF16, tag="bw")
                    # selected -> 0, not selected -> NEG
                    nc.vector.tensor_scalar(
                        bw[:], m1[:], 0.0, NEG,
                        op0=mybir.AluOpType.is_ge, op1=mybir.AluOpType.mult,
                    )
                    for k2 in range(2):
                        psB = pstp.tile([128, 128], BF16, tag="pst")
                        nc.tensor.transpose(
                            psB[:], bw[:, 128 * k2:128 * k2 + 128], ident
                        )
                        nc.vector.tensor_copy(
                            bwT[k2][:, h * 256 + 128 * i: h * 256 + 128 * i + 128],
                            psB[:],
                        )

            if dbg is not None:
                nc.sync.dma_start(dbg["bwt0"][:], bwT[0][:])
                nc.sync.dma_start(dbg["bwt1"][:], bwT[1][:])

            # ---------------- x load: transpose to padded channel-major ---------
            imgs = []
            for b in range(8):
                img = imgp.tile([128, PW * PW], BF16, tag=f"img{b}")
                nc.gpsimd.memset(img[:], 0.0)
                imgs.append(img)
            for b in range(8):
                xt = xp.tile([128, 1024], BF16, tag="xt")
                nc.gpsimd.memset(xt[:], 0.0)
                nc.sync.dma_start_transpose(
                    xt[0:48, :], x_in[:, 96 * b:96 * b + 48]
                )
                nc.sync.dma_start_transpose(
                    xt[64:112, :], x_in[:, 96 * b + 48:96 * b + 96]
                )
                dv = imgs[b][:].rearrange("p (a b) -> p a b", a=PW)[:, 2:34, 2:34]
                sv = xt[:].rearrange("p (a b) -> p a b", a=H)
                nc.vector.tensor_copy(dv, sv)

            # ---------------- conv: diag matmuls, 25 taps in PSUM ----------
            # ident_rep: identity tiled 25x along free; one broadcast-TT
            # builds all 25 taps' diagonal weights per (b, j)
            ident_rep = constp.tile([128, 25 * 128], BF16, tag="ident_rep")
            nc.vector.tensor_copy(
                ident_rep[:].rearrange("p (t q) -> p t q", t=25),
                ident[:, None, :].broadcast_to([128, 25, 128]),
            )
            q_chm, k_chm = [], []
            v_pix = [chmp.tile([128, 1024], BF16, tag=f"vpix{c8}", name=f"vpix{c8}") for c8 in range(8)]
            for b in range(8):
                img3 = imgs[b][:].rearrange("p (a b) -> p a b", a=PW)
                for j in range(3):
                    dwa = dwp.tile([128, 25 * 128], BF16, tag="dwa")
                    cw = b * 75 + j * 25
                    nc.vector.tensor_mul(
                        dwa[:].rearrange("p (t q) -> p t q", t=25),
                        ident_rep[:].rearrange("p (t q) -> p t q", t=25),
                        w_sb[:, cw:cw + 25][:, :, None].broadcast_to([128, 25, 128]),
                    )
                    pc = pcp.tile([128, 1024], F32, tag="pc")
                    for t in range(25):
                        dy, dx = t // 5, t % 5
                        nc.tensor.matmul(
                            pc[:, :512], dwa[:, 128 * t:128 * t + 128],
                            img3[:, dy:dy + 16, dx:dx + 32],
                            start=(t == 0), stop=(t == 24),
                        )
                        nc.tensor.matmul(
                            pc[:, 512:], dwa[:, 128 * t:128 * t + 128],
                            img3[:, dy + 16:dy + 32, dx:dx + 32],
                            start=(t == 0), stop=(t == 24),
                        )
                    bcol = b * 3 + j
                    dst = chmp.tile([128, 1024], BF16, tag=f"chm{j}_{b}")
                    nc.scalar.activation(
                        dst[:], pc[:],
                        mybir.ActivationFunctionType.Identity,
                        bias=bias_sb[:, bcol:bcol + 1], scale=1.0,
                    )
                    if j == 0:
                        q_chm.append(dst)
                    elif j == 1:
                        k_chm.append(dst)
                    else:
                        # V: rows 48/112 are all-ones via the conv bias
                        # (zero weights + bias=1) -> softmax denominator
                        for c8 in range(8):
                            psT = pstp.tile([128, 128], BF16, tag="pst")
                            nc.tensor.transpose(
                                psT[:], dst[:, 128 * c8:128 * c8 + 128], ident
                            )
                            nc.vector.tensor_copy(
                                v_pix[c8][:, 128 * b:128 * b + 128], psT[:]
                            )

            if dbg is not None:
                nc.sync.dma_start(dbg["qchm0"][:], q_chm[0][:])
                nc.sync.dma_start(dbg["kchm0"][:], k_chm[0][:])
                nc.sync.dma_start(dbg["vpix0"][:], v_pix[0][:])

            # ---------------- attention per head ----------------
            out_pix = [chmp.tile([128, 784], BF16, tag=f"opix{c8}", name=f"opix{c8}") for c8 in range(8)]
            for h in range(HEADS if phase >= 3 else 0):
                bh = h // 2
                p0 = 64 * (h % 2)
                pts = []
                for c in range(8):
                    cp = c % 4
                    k2 = cp // 2
                    rmat = r_even if (cp % 2 == 0) else r_odd
                    psS = pssp.tile([128, 1024], F32, tag="pss")
                    sl = bwT[k2][:, h * 256:h * 256 + 256]
                    for n0 in range(2):
                        nc.tensor.matmul(
                            psS[:, 512 * n0:512 * n0 + 512],
                            k_chm[bh][p0:p0 + 64, 128 * c:128 * c + 128],
                            q_chm[bh][p0:p0 + 64, 512 * n0:512 * n0 + 512],
                            start=True, stop=False,
                        )
                        # additive window-mask bias: rhs = bwT with the
                        # (qw -> qpx) expansion AP [(rq,16),(jq,0x2),(sq,1)]
                        rhs = bass.AP(
                            tensor=sl.tensor,
                            offset=sl.offset,
                            ap=[list(p) for p in sl.ap[:1]]
                            + [[16, 16], [0, 2], [1, 16]],
                        )
                        nc.tensor.matmul(
                            psS[:, 512 * n0:512 * n0 + 512],
                            rmat, rhs, start=False, stop=True,
                        )
                    pt = ptp.tile([128, 1024], BF16, tag="pt")
                    nc.scalar.activation(
                        pt[:], psS[:], mybir.ActivationFunctionType.Exp,
                        bias=0.0, scale=1.0,
                    )
                    pts.append(pt)
                    if dbg is not None and h == 0 and c == 0:
                        nc.sync.dma_start(dbg["pt00"][:], pt[:])
                psO = psop.tile([128, 1024], F32, tag="pso")
                for n0 in range(2):
                    for c in range(8):
                        nc.tensor.matmul(
                            psO[0:49, 512 * n0:512 * n0 + 512],
                            v_pix[c][:, 128 * bh + p0:128 * bh + p0 + 49],
                            pts[c][:, 512 * n0:512 * n0 + 512],
                            start=(c == 0), stop=(c == 7),
                        )
                oT = otp.tile([128, 1024], BF16, tag="ot")
                nc.vector.tensor_copy(oT[0:49, :], psO[0:49, :])
                if dbg is not None and h == 0:
                    nc.sync.dma_start(dbg["ot0"][:], oT[0:49, :])
                for c8 in range(8):
                    psT2 = pstp.tile([128, 128], BF16, tag="pst")
                    nc.tensor.transpose(
                        psT2[:, 0:49],
                        oT[0:49, 128 * c8:128 * c8 + 128],
                        ident[0:49, 0:49],
                    )
                    nc.vector.tensor_copy(
                        out_pix[c8][:, 49 * h:49 * h + 49], psT2[:, 0:49]
                    )

            # ---------------- normalize + quadrant-permuted store ----------
            for c8 in range(8 if phase >= 3 else 0):
                rc = finp.tile([128, 16], BF16, tag="rc")
                with nc.allow_low_precision(reason="softmax denom recip in bf16; 0.4% rel err ok"):
                    nc.vector.reciprocal(rc[:], out_pix[c8][:, 48:784:49])
                fin = finp.tile([128, 768], BF16, tag="fin")
                nc.vector.tensor_mul(
                    fin[:].rearrange("p (h c) -> p h c", h=16),
                    out_pix[c8][:].rearrange("p (h c) -> p h c", h=16)[:, :, 0:48],
                    rc[:, :, None].broadcast_to([128, 16, 48]),
                )
                nc.sync.dma_start(
                    o_out[128 * c8:128 * c8 + 128, :], fin[:]
                )
    return nc


def _build_program():
    from concourse import bacc, mybir

    nc = bacc.Bacc("TRN2", target_bir_lowering=False)
    F32 = mybir.dt.float32
    BF16 = mybir.dt.bfloat16

    x_in = nc.dram_tensor("x_in", [L, D], BF16, kind="ExternalInput")
    w_in = nc.dram_tensor("w_in", [128, 600], BF16, kind="ExternalInput")
    bias_in = nc.dram_tensor("bias_in", [128, 24], F32, kind="ExternalInput")
    idx_in = nc.dram_tensor("idx_in", [128, 256], BF16, kind="ExternalInput")
    consts_in = nc.dram_tensor("consts_in", [128, 640], BF16, kind="ExternalInput")
    o_out = nc.dram_tensor("o_out", [L, D], BF16, kind="ExternalOutput")

    _emit_device(nc, x_in, w_in, bias_in, idx_in, consts_in, o_out)
    nc.finalize()
    return nc


def _chan_of(b, p):
    if p < 48:
        return 96 * b + p
    if 64 <= p < 112:
        return 96 * b + 48 + (p - 64)
    return -1


def _host_prepare(conv_w, bn_gamma, bn_beta, bn_mean, bn_var):
    inv = bn_gamma / np.sqrt(bn_var + EPS)  # (3, 768)
    w_eff = conv_w[:, :, 0, :, :] * inv[:, :, None, None]  # (3, 768, 5, 5)
    b_eff = bn_beta - bn_mean * inv  # (3, 768)
    scale = float(D) ** -0.5
    w_eff = w_eff.copy()
    b_eff = b_eff.copy()
    w_eff[0] *= scale
    b_eff[0] *= scale

    w600 = np.zeros((128, 600), np.float32)
    bias24 = np.zeros((128, 24), np.float32)
    for b in range(8):
        for p in range(128):
            ch = _chan_of(b, p)
            if ch < 0:
                continue
            for j in range(3):
                w600[p, b * 75 + j * 25: b * 75 + j * 25 + 25] = w_eff[j, ch].reshape(25)
                bias24[p, b * 3 + j] = b_eff[j, ch]
        # V ones-rows (pad rows 48/112 have zero weights): bias 1.0 makes the
        # conv emit constant 1.0 there -> softmax denominator column in AV
        bias24[48, b * 3 + 2] = 1.0
        bias24[112, b * 3 + 2] = 1.0

    consts = np.zeros((128, 640), np.float32)
    consts[:, C_IOTA:C_IOTA + 256] = np.arange(256)[None, :]
    consts[:, C_ID:C_ID + 128] = np.eye(128)
    p = np.arange(128)
    kloc = 16 * (p // 32) + p % 16
    consts[:, C_REVEN:C_REVEN + 128] = (np.arange(128)[:, None] == kloc[None, :])
    consts[:, C_RODD:C_RODD + 128] = (np.arange(128)[:, None] == (64 + kloc)[None, :])
    return w600.astype(BF), bias24, consts.astype(BF)


def _topk_idx(gen_adj):
    # (nb, 16, 256, 256) -> idx_sb (nb, 128, 256) bf16 (set semantics; order free)
    nb = gen_adj.shape[0]
    flat = gen_adj.reshape(nb * HEADS * P2, P2)
    part = np.argpartition(flat, P2 - K, axis=-1)[:, P2 - K:]  # (nb*H*P2, 8)
    idx4 = part.reshape(nb, HEADS, 2, 128, K).transpose(0, 3, 1, 2, 4)
    return np.ascontiguousarray(idx4.reshape(nb, 128, 256)).astype(BF)


def kernel(x, noise, gen_adj, conv_w, bn_gamma, bn_beta, bn_mean, bn_var, sparsity):
    global LAST_EXEC_NS
    from concourse.bass_utils import run_bass_kernel_spmd

    assert int(sparsity) == K
    x = np.asarray(x, np.float32)
    gen_adj = np.asarray(gen_adj, np.float32)
    w600, bias24, consts = _host_prepare(
        np.asarray(conv_w, np.float32),
        np.asarray(bn_gamma, np.float32),
        np.asarray(bn_beta, np.float32),
        np.asarray(bn_mean, np.float32),
        np.asarray(bn_var, np.float32),
    )
    idx_sb = _topk_idx(gen_adj)
    x_bf = x.astype(BF)

    nc = _build_program()
    in_maps = []
    for bb in range(B):
        in_maps.append(
            {
                "x_in": np.ascontiguousarray(x_bf[bb]),
                "w_in": w600,
                "bias_in": bias24,
                "idx_in": np.ascontiguousarray(idx_sb[bb]),
                "consts_in": consts,
            }
        )

    trace = os.environ.get("KERNEL_TRACE", "0") == "1"
    res = run_bass_kernel_spmd(
        nc, in_maps, core_ids=list(range(B)), trace=trace
    )
    if trace:
        LAST_EXEC_NS = res.exec_time_ns
    if os.environ.get("KERNEL_TIME", "0") == "1":
        # second run hits the in-process PJRT executable cache; wall-time it
        import time as _time

        t0 = _time.time()
        res = run_bass_kernel_spmd(
            nc, in_maps, core_ids=list(range(B)), trace=False
        )
        LAST_EXEC_NS = int((_time.time() - t0) * 1e9)

    o = np.stack([np.asarray(r["o_out"], np.float32) for r in res.results])
    # quadrant permutation: out pixel (jq*16+r, iq*16+s) <- device row (iq*16+r, jq*16+s)
    o = o.reshape(B, 2, 16, 2, 16, D).transpose(0, 3, 2, 1, 4, 5)
    return np.ascontiguousarray(o.reshape(B, L, D))


if __name__ == "__main__":
    rng = np.random.default_rng(0)
    inputs = {
        "x": rng.standard_normal((B, L, D), dtype=np.float32),
        "noise": np.zeros((1,), np.float32),
        "gen_adj": rng.standard_normal((B, HEADS, P2, P2), dtype=np.float32),
        "conv_w": (rng.standard_normal((3, D, 1, 5, 5)) * 0.1).astype(np.float32),
        "bn_gamma": (1.0 + 0.1 * rng.standard_normal((3, D))).astype(np.float32),
        "bn_beta": (0.1 * rng.standard_normal((3, D))).astype(np.float32),
        "bn_mean": (0.1 * rng.standard_normal((3, D))).astype(np.float32),
        "bn_var": rng.uniform(0.5, 1.5, (3, D)).astype(np.float32),
        "sparsity": 8,
    }
    out = kernel(**inputs)
    print(out.shape, out.dtype, float(np.abs(out).max()))
